# revision 10
# baseline (speedup 1.0000x reference)
"""Trainium2 Bass kernel for LlamaAttention with MoE-routed LoRA (q/v).

Sharding: 8 cores = 4 batches x 2 head-halves. Core c handles batch b=c//2
and output-feature slice Jc = [1024*(c%2), 1024*(c%2)+1024) (8 of 16 heads).
Each core computes q/k/v projections (+ routed LoRA) for its slice, RoPE,
causal attention for its 8 heads, and a partial output projection
(contraction over its feature slice). Host sums the two partials per batch.

All device math is fp32. Matmul layouts:
  out[M,N] = lhsT[K,M].T @ rhs[K,N]   (K = partition/contraction dim)
qT/kT/vT are produced as [j(head-dim), s] so scores S^T[k,q] come out
directly in the layout whose softmax-normalizer can be applied without
transposing the probability matrix (l[q] via ones-matmul over partitions).
"""

import numpy as np

import concourse.bass as bass
import concourse.bacc as bacc
import concourse.mybir as mybir
import concourse.tile as tile
from concourse.bass_utils import run_bass_kernel_spmd

F32 = mybir.dt.float32
AF = mybir.ActivationFunctionType
ALU = mybir.AluOpType

# Problem constants (hardcoded per contract)
B, S, H, NH, HD = 4, 1024, 2048, 16, 128
R, E = 16, 7
T = E + 1
SCALING = 32.0 / R
TEMP = 1.0
BASE = 10000.0
P = 128
HT = H // P          # 16 h-chunks
G = 8                # heads per core
JW = G * HD          # 1024 features per core
SC = 2               # s-chunks of 512
NEG = -1.0e30

N_CORES = 8

_CACHE = {}


# --------------------------------------------------------------------------
# device program
# --------------------------------------------------------------------------

def _declare_io(nc):
    d = {}

    def inp(name, shape):
        d[name] = nc.dram_tensor(name, list(shape), F32, kind="ExternalInput").ap()

    inp("xT", (H, S))
    inp("WqT", (H, JW))
    inp("WkT", (H, JW))
    inp("WvT", (H, JW))
    inp("WoT", (JW, H))
    inp("AqT", (H, T * R))
    inp("BqT", (T * R, JW))
    inp("AvT", (H, T * R))
    inp("BvT", (T * R, JW))
    inp("MqT", (H, T))
    inp("VqT", (H, T))
    inp("MvT", (H, T))
    inp("VvT", (H, T))
    inp("COS", (P, S))
    inp("SIN", (P, S))
    inp("EXP8", (T, P))
    inp("ROT", (P, P))
    inp("MASK", (P, P))
    inp("IDT", (P, P))
    d["OUTA"] = nc.dram_tensor("OUTA", [S, H], F32, kind="ExternalOutput").ap()
    d["OUTB"] = nc.dram_tensor("OUTB", [S, H], F32, kind="ExternalOutput").ap()
    return d


def _emit_routing(nc, pools, dram, xt, side):
    """Gaussian-LL routing + top-2 renormalized softmax -> wcol [128,1]
    (weight per (task,rank) row, for scaling hcat)."""
    consts, small, psmm, pstr = (
        pools["consts"], pools["small"], pools["psmm"], pools["pstr"])

    mT = small.tile([P, HT, T], F32, tag=f"mT{side}", name=f"mT{side}")
    vT = small.tile([P, HT, T], F32, tag=f"vT{side}", name=f"vT{side}")
    nc.sync.dma_start(out=mT, in_=dram[f"M{side}T"].rearrange("(o p) t -> p o t", p=P))
    nc.sync.dma_start(out=vT, in_=dram[f"V{side}T"].rearrange("(o p) t -> p o t", p=P))

    feat = pools["feat"]
    # v' = v + 1e-6 ; lt = ln(2*pi*v') ; rv = 1/v'
    v1 = small.tile([P, HT, T], F32, tag=f"v1{side}", name=f"v1{side}")
    nc.vector.tensor_scalar_add(v1, vT, 1e-6)
    lt = small.tile([P, HT, T], F32, tag=f"lt{side}", name=f"lt{side}")
    nc.scalar.activation(lt, v1, AF.Ln, scale=float(2.0 * np.pi))
    rv = small.tile([P, HT, T], F32, tag=f"rv{side}", name=f"rv{side}")
    nc.vector.reciprocal(rv, v1)
    # d = m - feat (broadcast feat over t); su = d*d*rv + lt
    dd = small.tile([P, HT, T], F32, tag=f"dd{side}", name=f"dd{side}")
    nc.vector.tensor_tensor(dd, mT, feat[:, :, None].to_broadcast([P, HT, T]),
                            ALU.subtract)
    nc.vector.tensor_tensor(dd, dd, dd, ALU.mult)
    nc.vector.tensor_tensor(dd, dd, rv, ALU.mult)
    nc.vector.tensor_tensor(dd, dd, lt, ALU.add)
    # sum over HT (free, after axis swap) then over partitions via ones-matmul
    acc = small.tile([P, T], F32, tag=f"acc{side}", name=f"acc{side}")
    nc.vector.tensor_reduce(acc, dd.rearrange("p i t -> p t i"),
                            mybir.AxisListType.X, ALU.add)
    lg_ps = psmm.tile([1, T], F32, tag="mm", name=f"lgps{side}")
    nc.tensor.matmul(lg_ps, lhsT=pools["onescol"], rhs=acc, start=True, stop=True)
    lg = small.tile([1, T], F32, tag=f"lg{side}", name=f"lg{side}")
    nc.scalar.mul(lg, lg_ps, -0.5 / TEMP)

    # softmax over 8 with max subtraction + top-2 mask + renorm
    mx = small.tile([1, 1], F32, tag=f"mx{side}", name=f"mx{side}")
    nc.vector.tensor_reduce(mx, lg, mybir.AxisListType.X, ALU.max)
    nmx = small.tile([1, 1], F32, tag=f"nmx{side}", name=f"nmx{side}")
    nc.vector.tensor_scalar_mul(nmx, mx, -1.0)
    eq = small.tile([1, T], F32, tag=f"eq{side}", name=f"eq{side}")
    nc.vector.tensor_scalar(eq, lg, mx, None, ALU.is_ge)
    nc.vector.tensor_scalar_mul(eq, eq, 1e30)
    lg2 = small.tile([1, T], F32, tag=f"lg2{side}", name=f"lg2{side}")
    nc.vector.tensor_tensor(lg2, lg, eq, ALU.subtract)
    mx2 = small.tile([1, 1], F32, tag=f"mx2{side}", name=f"mx2{side}")
    nc.vector.tensor_reduce(mx2, lg2, mybir.AxisListType.X, ALU.max)
    msk = small.tile([1, T], F32, tag=f"msk{side}", name=f"msk{side}")
    nc.vector.tensor_scalar(msk, lg, mx2, None, ALU.is_ge)
    ee = small.tile([1, T], F32, tag=f"ee{side}", name=f"ee{side}")
    nc.scalar.activation(ee, lg, AF.Exp, bias=nmx[:, 0:1])
    nc.vector.tensor_tensor(ee, ee, msk, ALU.mult)
    ssum = small.tile([1, 1], F32, tag=f"ss{side}", name=f"ss{side}")
    nc.vector.tensor_reduce(ssum, ee, mybir.AxisListType.X, ALU.add)
    rs = small.tile([1, 1], F32, tag=f"rs{side}", name=f"rs{side}")
    nc.vector.reciprocal(rs, ssum)
    w = small.tile([1, T], F32, tag=f"w{side}", name=f"w{side}")
    nc.vector.tensor_scalar(w, ee, rs, None, ALU.mult)

    # w [1,8] -> wT [8,1] -> wcol [128,1] = EXP8.T @ wT
    wt_ps = pstr.tile([T, 1], F32, tag="tr", name=f"wtps{side}")
    nc.tensor.transpose(wt_ps, w, pools["idt"][:1, :1])
    wt = small.tile([T, 1], F32, tag=f"wt{side}", name=f"wt{side}")
    nc.vector.tensor_copy(wt, wt_ps)
    wc_ps = pstr.tile([P, 1], F32, tag="tr", name=f"wcps{side}")
    nc.tensor.matmul(wc_ps, lhsT=pools["exp8"], rhs=wt, start=True, stop=True)
    wcol = small.tile([P, 1], F32, tag=f"wcol{side}", name=f"wcol{side}")
    nc.vector.tensor_copy(wcol, wc_ps)
    return wcol


def _emit_hcat(nc, pools, dram, xt, wcol, side):
    """hcatT [tr=128, s=1024] = (Acat @ x^T) scaled per-row by wcol."""
    wstream, psmm = pools["wstream"], pools["psmm"]
    h_sb = pools["misc"].tile([P, S], F32, tag=f"hcat{side}", name=f"hcat{side}")
    achunks = []
    for i in range(HT):
        a_sb = wstream.tile([P, P], F32, tag="wch", name=f"a{side}{i}")
        nc.sync.dma_start(out=a_sb, in_=dram[f"A{side}T"][i * P:(i + 1) * P, :])
        achunks.append(a_sb)
    for sc in range(SC):
        ps = psmm.tile([P, 512], F32, tag="mm", name=f"hps{side}{sc}")
        for i in range(HT):
            nc.tensor.matmul(ps, lhsT=achunks[i], rhs=xt[:, i, sc * 512:(sc + 1) * 512],
                             start=(i == 0), stop=(i == HT - 1))
        nc.vector.tensor_scalar(h_sb[:, sc * 512:(sc + 1) * 512], ps, wcol[:, 0:1],
                                None, ALU.mult)
    return h_sb


def _emit_program(tc, dram):
    nc = tc.nc
    import contextlib
    ctx = contextlib.ExitStack()
    with ctx:
        pools = {}
        consts = ctx.enter_context(tc.tile_pool(name="consts", bufs=1))
        big = ctx.enter_context(tc.tile_pool(name="big", bufs=1))
        misc = ctx.enter_context(tc.tile_pool(name="misc", bufs=1))
        small = ctx.enter_context(tc.tile_pool(name="small", bufs=1))
        wstream = ctx.enter_context(tc.tile_pool(name="wstream", bufs=18))
        wide = ctx.enter_context(tc.tile_pool(name="wide", bufs=8))
        tmp = ctx.enter_context(tc.tile_pool(name="tmp", bufs=2))
        ptile = ctx.enter_context(tc.tile_pool(name="ptile", bufs=2))
        vnat_p = ctx.enter_context(tc.tile_pool(name="vnat", bufs=2))
        qkv = ctx.enter_context(tc.tile_pool(name="qkv", bufs=2))
        psmm = ctx.enter_context(tc.tile_pool(name="psmm", bufs=2, space="PSUM"))
        pstr = ctx.enter_context(tc.tile_pool(name="pstr", bufs=2, space="PSUM"))
        psst = ctx.enter_context(tc.tile_pool(name="psst", bufs=2, space="PSUM"))
        pspv = ctx.enter_context(tc.tile_pool(name="pspv", bufs=1, space="PSUM"))
        psl = ctx.enter_context(tc.tile_pool(name="psl", bufs=1, space="PSUM"))
        pools.update(consts=consts, misc=misc, small=small, wstream=wstream,
                     wide=wide, tmp=tmp, ptile=ptile, qkv=qkv,
                     psmm=psmm, pstr=pstr, psst=psst, pspv=pspv, psl=psl)

        # ---- constants
        exp8 = consts.tile([T, P], F32, tag="exp8", name="exp8")
        nc.sync.dma_start(out=exp8, in_=dram["EXP8"])
        rot = consts.tile([P, P], F32, tag="rot", name="rot")
        nc.sync.dma_start(out=rot, in_=dram["ROT"])
        maskst = consts.tile([P, P], F32, tag="maskst", name="maskst")
        nc.sync.dma_start(out=maskst, in_=dram["MASK"])
        idt = consts.tile([P, P], F32, tag="idt", name="idt")
        nc.sync.dma_start(out=idt, in_=dram["IDT"])
        onescol = consts.tile([P, 1], F32, tag="onescol", name="onescol")
        nc.vector.memset(onescol, 1.0)
        onesrow = consts.tile([1, P], F32, tag="onesrow", name="onesrow")
        nc.vector.memset(onesrow, 1.0)
        cos = consts.tile([P, S], F32, tag="cos", name="cos")
        nc.sync.dma_start(out=cos, in_=dram["COS"])
        sin = consts.tile([P, S], F32, tag="sin", name="sin")
        nc.sync.dma_start(out=sin, in_=dram["SIN"])
        pools["exp8"], pools["idt"] = exp8, idt
        pools["onescol"], pools["onesrow"] = onescol, onesrow

        # ---- x^T resident [128, 16, 1024]
        xt = big.tile([P, HT, S], F32, tag="xt", name="xt")
        nc.sync.dma_start(out=xt, in_=dram["xT"].rearrange("(o p) s -> p o s", p=P))

        # ---- routing features: feat[p, i] = mean_s xT[p, i, s]
        feat = misc.tile([P, HT], F32, tag="feat", name="feat")
        for i in range(HT):
            nc.vector.tensor_reduce(feat[:, i:i + 1], xt[:, i, :],
                                    mybir.AxisListType.X, ALU.add)
        nc.vector.tensor_scalar_mul(feat, feat, 1.0 / S)
        pools["feat"] = feat

        wcol_q = _emit_routing(nc, pools, dram, xt, "q")
        wcol_v = _emit_routing(nc, pools, dram, xt, "v")
        hq = _emit_hcat(nc, pools, dram, xt, wcol_q, "q")
        hv = _emit_hcat(nc, pools, dram, xt, wcol_v, "v")

        # ---- attention accumulator for a 4-head group (consumed by Wo pass)
        GH = G // 2

        def wo_pass(attnT, out_ap, tagp):
            for ic in range(H // 512):
                wo_sb = []
                for g in range(GH):
                    wo = wide.tile([P, 512], F32, tag="wo", name=f"wo{tagp}{ic}{g}")
                    nc.sync.dma_start(
                        out=wo,
                        in_=dram["WoT"][(tagp * GH + g) * P:(tagp * GH + g + 1) * P,
                                        ic * 512:(ic + 1) * 512])
                    wo_sb.append(wo)
                for stt in range(S // P):
                    ps = psmm.tile([P, 512], F32, tag="mm", name=f"po{tagp}{ic}{stt}")
                    for g in range(GH):
                        nc.tensor.matmul(ps, lhsT=attnT[:, g, stt * P:(stt + 1) * P],
                                         rhs=wo_sb[g], start=(g == 0),
                                         stop=(g == GH - 1), skip_group_check=True)
                    ev = tmp.tile([P, 512], F32, tag="ev", name=f"ev{tagp}{ic}{stt}")
                    nc.vector.tensor_copy(ev, ps)
                    nc.sync.dma_start(
                        out=out_ap[stt * P:(stt + 1) * P, ic * 512:(ic + 1) * 512],
                        in_=ev)

        inv_sqrt_hd = 1.0 / float(np.sqrt(HD))
        attnT = None

        for g in range(G):
            if g % GH == 0:
                attnT = big.tile([P, GH, S], F32, tag="attnT", bufs=2,
                                 name=f"attnT{g}")
            # ---------- project qT_g, kT_g, vT_g  [128 d, 1024 s]
            heads = {}
            for nm, wname, lora in (("q", "WqT", ("BqT", hq)),
                                    ("k", "WkT", None),
                                    ("v", "WvT", ("BvT", hv))):
                dst = qkv.tile([P, S], F32, tag=f"{nm}t", name=f"{nm}t{g}")
                wchunks = []
                for h in range(HT):
                    wch = wstream.tile([P, P], F32, tag="wch", name=f"w{nm}{g}{h}")
                    nc.sync.dma_start(
                        out=wch,
                        in_=dram[wname][h * P:(h + 1) * P, g * P:(g + 1) * P])
                    wchunks.append(wch)
                bchunk = None
                if lora is not None:
                    bchunk = wstream.tile([P, P], F32, tag="wch", name=f"b{nm}{g}")
                    nc.sync.dma_start(
                        out=bchunk, in_=dram[lora[0]][:, g * P:(g + 1) * P])
                for sc in range(SC):
                    ps = psmm.tile([P, 512], F32, tag="mm", name=f"p{nm}{g}{sc}")
                    for h in range(HT):
                        nc.tensor.matmul(
                            ps, lhsT=wchunks[h], rhs=xt[:, h, sc * 512:(sc + 1) * 512],
                            start=(h == 0),
                            stop=(h == HT - 1 and lora is None))
                    if lora is not None:
                        nc.tensor.matmul(
                            ps, lhsT=bchunk,
                            rhs=lora[1][:, sc * 512:(sc + 1) * 512],
                            start=False, stop=True)
                    nc.vector.tensor_copy(dst[:, sc * 512:(sc + 1) * 512], ps)
                heads[nm] = dst
            qt_g, kt_g, vt_g = heads["q"], heads["k"], heads["v"]

            # ---------- RoPE on qT_g, kT_g (in place)
            for x in (qt_g, kt_g):
                for sc in range(SC):
                    sl = slice(sc * 512, (sc + 1) * 512)
                    rp = psmm.tile([P, 512], F32, tag="mm", name=f"rp{g}{sc}")
                    nc.tensor.matmul(rp, lhsT=rot, rhs=x[:, sl], start=True, stop=True)
                    t1 = tmp.tile([P, 512], F32, tag="t1", name=f"t1{g}{sc}")
                    nc.vector.tensor_tensor(t1, x[:, sl], cos[:, sl], ALU.mult)
                    t2 = tmp.tile([P, 512], F32, tag="t2", name=f"t2{g}{sc}")
                    nc.vector.tensor_tensor(t2, rp, sin[:, sl], ALU.mult)
                    nc.vector.tensor_tensor(x[:, sl], t1, t2, ALU.add)

            # ---------- V natural layout: vnat[k-part, kt, d]
            vnat = vnat_p.tile([P, G, P], F32, tag="vnat", name=f"vnat{g}")
            for kt in range(G):
                trp = pstr.tile([P, P], F32, tag="tr", name=f"trp{g}{kt}")
                nc.tensor.transpose(trp, vt_g[:, kt * P:(kt + 1) * P], idt)
                nc.vector.tensor_copy(vnat[:, kt, :], trp)

            # ---------- attention for head g
            for qc in range(SC):
                ktmax = min(G - 1, (qc * 512 + 511) // P)
                pv = pspv.tile([P, 512], F32, tag="pv", name=f"pv{g}{qc}")
                lps = psl.tile([1, 512], F32, tag="l", name=f"l{g}{qc}")
                for kt in range(ktmax + 1):
                    off = max(0, kt * P - qc * 512)
                    st = psst.tile([P, 512], F32, tag="st", name=f"st{g}{qc}{kt}")
                    nc.tensor.matmul(
                        st[:, off:], lhsT=kt_g[:, kt * P:(kt + 1) * P],
                        rhs=qt_g[:, qc * 512 + off:(qc + 1) * 512],
                        start=True, stop=True, skip_group_check=True)
                    if kt * P >= qc * 512:  # diagonal block needs intra-block mask
                        nc.vector.tensor_tensor(st[:, off:off + P], st[:, off:off + P],
                                                maskst, ALU.add)
                    pt = ptile.tile([P, 512], F32, tag="pt", name=f"pt{g}{qc}{kt}")
                    nc.scalar.activation(pt[:, off:], st[:, off:], AF.Exp,
                                         scale=inv_sqrt_hd)
                    nc.tensor.matmul(lps[:, off:], lhsT=onescol, rhs=pt[:, off:],
                                     start=(kt == 0), stop=(kt == ktmax),
                                     skip_group_check=True)
                    nc.tensor.matmul(pv[:, off:], lhsT=vnat[:, kt, :], rhs=pt[:, off:],
                                     start=(kt == 0), stop=(kt == ktmax),
                                     skip_group_check=True)
                rec = small.tile([1, 512], F32, tag="rec", name=f"rec{g}{qc}")
                nc.vector.reciprocal(rec, lps)
                lbc_ps = pstr.tile([P, 512], F32, tag="tr", name=f"lbc{g}{qc}")
                nc.tensor.matmul(lbc_ps, lhsT=onesrow, rhs=rec, start=True, stop=True,
                                 skip_group_check=True)
                lbc = tmp.tile([P, 512], F32, tag="lbc", name=f"lbcs{g}{qc}")
                nc.vector.tensor_copy(lbc, lbc_ps)
                nc.vector.tensor_tensor(attnT[:, g % GH, qc * 512:(qc + 1) * 512],
                                        pv, lbc, ALU.mult)

            if g % GH == GH - 1:  # output projection for the finished 4-head group
                wo_pass(attnT, dram["OUTA" if g < GH else "OUTB"], g // GH)


def build_program():
    if "nc" in _CACHE:
        return _CACHE["nc"]
    nc = bacc.Bacc("TRN2", target_bir_lowering=False, debug=False,
                   num_devices=N_CORES)
    dram = _declare_io(nc)
    with tile.TileContext(nc) as tc:
        _emit_program(tc, dram)
    nc.finalize()
    _CACHE["nc"] = nc
    return nc


# --------------------------------------------------------------------------
# host-side sharding / constants
# --------------------------------------------------------------------------

def _rope_tables():
    inv = 1.0 / (BASE ** (np.arange(0, HD, 2, dtype=np.float64) / HD))
    t = np.arange(S, dtype=np.float64)
    emb = np.concatenate([np.outer(t, inv), np.outer(t, inv)], axis=-1)
    return (np.cos(emb).astype(np.float32), np.sin(emb).astype(np.float32))


def _consts():
    exp8 = np.zeros((T, P), np.float32)
    for t in range(T):
        exp8[t, t * R:(t + 1) * R] = 1.0
    rot = np.zeros((P, P), np.float32)     # ROT[k, d'] = R[d', k]
    half = HD // 2
    for dp in range(half):
        rot[dp + half, dp] = -1.0          # rot(q)[d'] = -q[d'+64], d' < 64
    for dp in range(half, HD):
        rot[dp - half, dp] = 1.0           # rot(q)[d'] =  q[d'-64], d' >= 64
    mask = np.zeros((P, P), np.float32)    # S^T block mask: k(part) <= q(free)
    for k in range(P):
        mask[k, :k] = NEG
    idt = np.eye(P, dtype=np.float32)
    return exp8, rot, mask, idt


def make_in_maps(inputs):
    x = np.asarray(inputs["hidden_states"], np.float32)
    pos = np.asarray(inputs["position_ids"])
    cos_tab, sin_tab = _rope_tables()
    exp8, rot, mask, idt = _consts()

    def cat_a(cur, prev):   # [T*R, H]
        return np.concatenate([cur[None], prev], axis=0).reshape(T * R, H)

    def cat_bt(cur, prev, js):  # [T*R, JW], SCALING folded in
        stk = np.concatenate([cur[js][None].transpose(0, 2, 1),
                              prev[:, js, :].transpose(0, 2, 1)], axis=0)
        return (stk.reshape(T * R, JW) * SCALING).astype(np.float32)

    AqT = np.ascontiguousarray(cat_a(inputs["loraA_q"], inputs["prevA_q"]).T)
    AvT = np.ascontiguousarray(cat_a(inputs["loraA_v"], inputs["prevA_v"]).T)
    MqT = np.ascontiguousarray(np.asarray(inputs["means_q"], np.float32).T)
    VqT = np.ascontiguousarray(np.asarray(inputs["vars_q"], np.float32).T)
    MvT = np.ascontiguousarray(np.asarray(inputs["means_v"], np.float32).T)
    VvT = np.ascontiguousarray(np.asarray(inputs["vars_v"], np.float32).T)

    in_maps = []
    for c in range(N_CORES):
        b, hh = divmod(c, 2)
        js = slice(JW * hh, JW * hh + JW)
        cos_b = cos_tab[pos[b]]            # [S, HD] gather
        sin_b = sin_tab[pos[b]]
        m = {
            "xT": np.ascontiguousarray(x[b].T),
            "WqT": np.ascontiguousarray(np.asarray(inputs["Wq"], np.float32)[js].T),
            "WkT": np.ascontiguousarray(np.asarray(inputs["Wk"], np.float32)[js].T),
            "WvT": np.ascontiguousarray(np.asarray(inputs["Wv"], np.float32)[js].T),
            "WoT": np.ascontiguousarray(np.asarray(inputs["Wo"], np.float32)[:, js].T),
            "AqT": AqT, "AvT": AvT,
            "BqT": cat_bt(np.asarray(inputs["loraB_q"], np.float32),
                          np.asarray(inputs["prevB_q"], np.float32), js),
            "BvT": cat_bt(np.asarray(inputs["loraB_v"], np.float32),
                          np.asarray(inputs["prevB_v"], np.float32), js),
            "MqT": MqT, "VqT": VqT, "MvT": MvT, "VvT": VvT,
            "COS": np.ascontiguousarray(cos_b.T),
            "SIN": np.ascontiguousarray(sin_b.T),
            "EXP8": exp8, "ROT": rot, "MASK": mask, "IDT": idt,
        }
        in_maps.append(m)
    return in_maps


def combine_outputs(results):
    out = np.empty((B, S, H), np.float32)
    for b in range(B):
        out[b] = (results[2 * b]["OUTA"] + results[2 * b]["OUTB"]
                  + results[2 * b + 1]["OUTA"] + results[2 * b + 1]["OUTB"])
    return out


def kernel(**inputs):
    nc = build_program()
    in_maps = make_in_maps(inputs)
    res = run_bass_kernel_spmd(nc, in_maps, core_ids=list(range(N_CORES)))
    return combine_outputs(res.results)
